# revision 45
# baseline (speedup 1.0000x reference)
"""TRN2 Bass kernel for nn_NeuralODE_57999238365256.

Single-trajectory Neural ODE (Tsit5, adaptive with bounded attempts).  The
adaptive controller's control flow is provably fixed for this instance
(DT0=0.1 always exceeds the save interval, the first attempt of every
interval lands exactly on t1 and is always accepted, attempts after `done`
are bit-exact no-ops), so the device runs 99 fixed Tsit5 steps with
dt_i = fl(ts[i+1]-ts[i]) and FSAL reuse; the error-norm/controller math is
not computed at all.

Algebraic reformulation (the big win vs the straightforward kernel): every
k_j = W_out@h_j + b_out is only ever consumed by linear maps, so the
3072-dim state never needs to be materialized on the device:
  - G := W_in@W_out (768x768) and c := W_in@b_out are host-precomputed.
  - w' := W_in@y + b_in is tracked inductively:
        w'_{n+1} = w' + dt * sum_j B_j g'_j,  g'_j := G@h_j + c.
  - stage pre-activations: a_s = w' + dt * sum_j A_sj g'_j  (no W_in apply)
  - y is materialized only in its first 1536 components (needed for the
    output): ytop += dt*(W_out_top @ sum_j B_j h_j) + dt*(sum B)*b_out_top.
This cuts the PE weight traffic per step from 6 evals x (W_in,3xW_h,W_out)
= 2376 128x128 tiles to 6 x (3xW_h + G) + W_out_top = 936 tiles.

Precision: compensated fp16 everywhere (W = W1 + W2/2^10, x = x1 + x2,
accumulating W1@x1 + W1@x2 + W2s@x1s in fp32 PSUM => ~2^-22 per-product
error, end-to-end max-rel ~1e-3 vs the CPU reference).  The x1/x2 passes
share one weight load: x1,x2 are interleaved pairs in SBUF and ride a
single N=2 matmul per weight tile; psum column pairs are merged by DVE.

Sharding: the trajectory is inherently sequential (every matvec depends on
the previous one) and cross-core collectives have a ~10us floor -- far too
slow for ~2.5us layer matvecs.  Everything runs SBUF-resident on core 0.
"""

import numpy as np

HIDDEN, TOP, NSTEPS = 768, 1536, 100
CH, CT = HIDDEN // 128, TOP // 128  # 6, 12
N_SCAL = 22  # 15 A*dt, 6 B*dt, dt

A_COEF = [
    [0.161],
    [-0.008480655492356989, 0.335480655492357],
    [2.8971530571054935, -6.359448489975075, 4.3622954328695815],
    [5.325864828439257, -11.748883564062828, 7.4955393428898365,
     -0.09249506636175525],
    [5.86145544294642, -12.92096931784711, 8.159367898576159,
     -0.071584973281401, -0.028269050394068383],
]
B_COEF = [0.09646076681806523, 0.01, 0.4798896504144996, 1.379008574103742,
          -3.290069515436081, 2.324710524099774]


def _col_layout(v):
    d = v.shape[-1]
    return v.reshape(*v.shape[:-1], d // 128, 128).swapaxes(-1, -2)


def _uncol_layout(m):
    return m.swapaxes(-1, -2).reshape(*m.shape[:-2], -1)


def _lhsT_layout(W):
    out_d, in_d = W.shape
    Wt = np.ascontiguousarray(W.T)
    return np.ascontiguousarray(
        Wt.reshape(in_d // 128, 128, out_d).transpose(1, 0, 2).reshape(
            128, (in_d // 128) * out_d))


def _make_scal_table():
    ts = np.linspace(0.0, 1.0, NSTEPS).astype(np.float32)
    tab = np.zeros((NSTEPS - 1, N_SCAL), np.float32)
    for i in range(NSTEPS - 1):
        dt = np.float64(np.float32(ts[i + 1] - ts[i]))
        vals = []
        for row in A_COEF:
            vals += [np.float32(dt * c) for c in row]
        vals += [np.float32(dt * c) for c in B_COEF]
        vals.append(np.float32(dt))
        tab[i] = vals
    return tab


def _prep_host_inputs(inputs):
    f16, f32, f64 = np.float16, np.float32, np.float64
    f = {}

    def wsplit(name, W, fp8=False):
        import ml_dtypes
        L = _lhsT_layout(np.asarray(W, f32))
        W1 = L.astype(f16)
        R = L - W1.astype(f32)
        if fp8:
            # e4m3 residual, scaled 2^14 into normal range; the rhs side
            # carries the 2^-14 (fp16 handles that subnormal scale exactly)
            f[name + "_2"] = (R * 16384.0).astype(ml_dtypes.float8_e4m3)
        else:
            f[name + "_2"] = (R * 1024.0).astype(f16)
        f[name + "_1"] = W1

    W_in = np.asarray(inputs["W_in"], f32)
    W_out = np.asarray(inputs["W_out"], f32)
    W_hid = np.asarray(inputs["W_hid"], f32)
    b_in = np.asarray(inputs["b_in"], f32)
    b_out = np.asarray(inputs["b_out"], f32)
    b_hid = np.asarray(inputs["b_hid"], f32)
    y0 = np.asarray(inputs["y0"], f32)

    G = (W_in.astype(f64) @ W_out.astype(f64)).astype(f32)
    cvec = (W_in.astype(f64) @ b_out.astype(f64)).astype(f32)
    w0 = (W_in.astype(f64) @ y0.astype(f64) + b_in.astype(f64)).astype(f32)
    kappa = np.sum(np.asarray(B_COEF, f64))

    for i in range(3):
        wsplit(f"Wt_h{i}", W_hid[i])
    wsplit("Wt_g", G)
    wsplit("Wt_o", W_out[:TOP])

    for i in range(3):
        f[f"b_h{i}_c"] = np.ascontiguousarray(_col_layout(b_hid[i]))
    f["cvec_c"] = np.ascontiguousarray(_col_layout(cvec))
    f["boK_c"] = np.ascontiguousarray(
        _col_layout((kappa * b_out[:TOP].astype(f64)).astype(f32)))
    f["w0_c"] = np.ascontiguousarray(_col_layout(w0))
    f["ytop0_c"] = np.ascontiguousarray(_col_layout(y0[:TOP]))
    epsc = _col_layout(np.asarray(inputs["eps"], f32))
    f["eps_c"] = np.ascontiguousarray(
        epsc.transpose(1, 0, 2).reshape(128, NSTEPS * CH))
    tab = _make_scal_table()
    f["scal"] = np.ascontiguousarray(
        np.broadcast_to(tab.reshape(1, -1), (128, tab.size)))
    return f


_CACHE = {}


def _build_kernel(repeat=1, variant=""):
    # variant: "" (real kernel) | "now2" (skip W2 pass; timing only)
    #          | "nope" (skip all matmul streams; timing only)
    #          | "nochain" (constant x12/x1s; streams unchained; timing only)
    import concourse.bass as bass
    import concourse.bacc as bacc
    import concourse.tile as tile
    import concourse.mybir as mybir
    from contextlib import ExitStack

    F32 = mybir.dt.float32
    F16 = mybir.dt.float16
    F8 = mybir.dt.float8e4
    n_steps = NSTEPS - 1

    # Force Exp AND Ln onto the single common activation-table set
    # (natural_log_exp_and_others, index 6).  Left to itself the table-load
    # pass picks exp_and_others for Exp and natural_log for Ln, emitting
    # TWO ~1.3us table loads inside the loop per softplus (~48/step) —
    # that was the dominant cost of the whole kernel.  Indices of the
    # other sets are preserved (emptied, not removed) because the emitted
    # act_func_set_id indexes this list.
    orig_tables = bacc.get_activation_tables

    def _one_table(arch):
        t = orig_tables(arch)
        return {k: (v if k == "natural_log_exp_and_others" else set())
                for k, v in t.items()}

    bacc.get_activation_tables = _one_table
    nc = bacc.Bacc("TRN2", target_bir_lowering=False, debug=False,
                   enable_asserts=False, num_devices=1)
    dram = {}

    def din(name, shape, dt=F32):
        dram[name] = nc.dram_tensor(name, list(shape), dt,
                                    kind="ExternalInput").ap()

    for suf in ("_1", "_2"):
        for i in range(3):
            din(f"Wt_h{i}" + suf, [128, CH * HIDDEN], F16)
        din("Wt_g" + suf, [128, CH * HIDDEN], F16)
        din("Wt_o" + suf, [128, CH * TOP], F16)
    for i in range(3):
        din(f"b_h{i}_c", [128, CH])
    din("cvec_c", [128, CH])
    din("boK_c", [128, CT])
    din("w0_c", [128, CH])
    din("ytop0_c", [128, CT])
    din("eps_c", [128, NSTEPS * CH])
    din("scal", [128, n_steps * N_SCAL])
    out_ap = nc.dram_tensor("out_c", [128, NSTEPS * CH], F32,
                            kind="ExternalOutput").ap()

    with tile.TileContext(nc) as tc, ExitStack() as ctx:
        persist = ctx.enter_context(tc.tile_pool(name="persist", bufs=1))
        psum_p = ctx.enter_context(tc.tile_pool(name="ps", bufs=2,
                                                space="PSUM"))
        psum_big = ctx.enter_context(tc.tile_pool(name="psb", bufs=2,
                                                  space="PSUM"))
        psact = ctx.enter_context(tc.tile_pool(name="psact", bufs=1,
                                               space="PSUM"))

        sb = {}
        for name in dram:
            if name in ("w0_c", "ytop0_c"):
                continue
            t = persist.tile(list(dram[name].shape), dram[name].dtype,
                             tag=name, name=name + "_sb")
            nc.sync.dma_start(t[:], dram[name])
            sb[name] = t

        w = persist.tile([128, CH], F32, tag="w", name="w")
        wacc = persist.tile([128, CH], F32, tag="wacc", name="wacc")
        ytop = persist.tile([128, CT], F32, tag="ytop", name="ytop")
        nc.sync.dma_start(w[:], dram["w0_c"])
        nc.sync.dma_start(ytop[:], dram["ytop0_c"])
        gs = [persist.tile([128, CH], F32, tag=f"g{j}", name=f"g{j}")
              for j in range(7)]
        S = persist.tile([128, CH], F32, tag="S", name="S")
        out_sb = persist.tile([128, NSTEPS * CH], F32, tag="out_sb",
                              name="out_sb")
        x12 = persist.tile([128, CH, 2], F16, tag="x12", name="x12")
        x1s = persist.tile([128, CH], F16, tag="x1s", name="x1s")
        S12 = persist.tile([128, CH, 2], F16, tag="S12", name="S12")
        S1s = persist.tile([128, CH], F16, tag="S1s", name="S1s")
        et = persist.tile([128, CH], F32, tag="et", name="et")
        etp = psact.tile([128, CH], F32, tag="etp", name="etp")
        efp = psact.tile([128, CH], F32, tag="efp", name="efp")
        hf = persist.tile([128, CH], F32, tag="hf", name="hf")
        z6 = persist.tile([128, CH], F32, tag="z6", name="z6")
        zin = persist.tile([128, CH], F32, tag="zin", name="zin")
        z12 = persist.tile([128, CT], F32, tag="z12", name="z12")
        zc = persist.tile([128, CH], F32, tag="zc", name="zc")

        st_ab = [persist.tile([128, N_SCAL], F32, tag=f"st{a}", name=f"st{a}")
                 for a in range(2)]

        AluOp = mybir.AluOpType
        ActFn = mybir.ActivationFunctionType

        cur_st = [st_ab[0]]

        def sc(idx):
            # staged scal[:, idx] as a [128,1] broadcast scalar
            return cur_st[0][:, idx:idx + 1]

        def stt(out, in0, scalar, in1, op0=None, op1=None):
            nc.vector.scalar_tensor_tensor(
                out, in0, scalar, in1,
                op0 or AluOp.mult, op1 or AluOp.add)

        def mv(wname, xp, xs, ck, cm, pool):
            """Compensated matvec: psum[:,m,0]+psum[:,m,1] = W@x.

            xp: [128, ck, 2] f16 interleaved (x1, x2); xs: [128, ck] f16
            (x1 * 2^-10).  W1 rides one N=2 matmul per tile (single weight
            load for both halves); W2s accumulates onto column 0.
            """
            w1, w2 = sb[wname + "_1"], sb[wname + "_2"]
            ps = pool.tile([128, cm, 2], F32, name="mv_ps")
            if variant == "nope":
                nc.vector.tensor_copy(ps[:, :, 0], xp[:, :cm, 0])
                nc.vector.tensor_copy(ps[:, :, 1], xp[:, :cm, 0])
                return ps
            for m in range(cm):
                base = m * 128
                for k in range(ck):
                    nc.tensor.matmul(
                        ps[:, m, :],
                        w1[:, k * (cm * 128) + base:
                           k * (cm * 128) + base + 128],
                        xp[:, k, :],
                        start=(k == 0),
                        stop=(variant == "now2" and k == ck - 1),
                        skip_group_check=True)
                if variant == "now2":
                    continue
                for k in range(ck):
                    nc.tensor.matmul(
                        ps[:, m, 0:1],
                        w2[:, k * (cm * 128) + base:
                           k * (cm * 128) + base + 128],
                        xs[:, k:k + 1],
                        start=False, stop=(k == ck - 1),
                        skip_group_check=True)
            return ps

        def split_x(src_f32, xp, xs, c, ss=2.0 ** -10):
            """xp[:,:,0]=f16(src), xp[:,:,1]=f16(src-hi), xs=hi*ss."""
            nc.vector.tensor_copy(xp[:, :, 0], src_f32[:, :c])
            nc.vector.tensor_tensor(xp[:, :, 1], src_f32[:, :c], xp[:, :, 0],
                                    AluOp.subtract)
            nc.vector.tensor_scalar(xs[:, :c], xp[:, :, 0], ss, None,
                                    AluOp.mult)

        def softplus_from(src):
            """hf = ln(1+exp(src)); ef routed via PSUM (ScE's fast port)."""
            nc.scalar.activation(efp[:], src, ActFn.Exp)
            nc.scalar.activation(hf[:], efp[:], ActFn.Ln, bias=1.0)

        def layer(ps, bias_t, ss):
            """psum pair + bias -> softplus -> split into x12/x1s."""
            if variant == "nochain":
                return
            stt(et[:], ps[:, :, 0], 0.0, bias_t, AluOp.add, AluOp.add)
            stt(etp[:], ps[:, :, 1], 0.0, et[:], AluOp.add, AluOp.add)
            softplus_from(etp[:])
            split_x(hf, x12, x1s, CH, ss)

        def hidden_chain():
            """3 hidden layers; consumes x12/x1s holding softplus(a_s)."""
            for li in range(3):
                ps = mv(f"Wt_h{li}", x12, x1s, CH, CH, psum_p)
                layer(ps, sb[f"b_h{li}_c"][:], 2.0 ** -10)

        def g_stage(dst):
            """dst = G@h + c from current x12/x1s."""
            ps = mv("Wt_g", x12, x1s, CH, CH, psum_p)
            if variant == "nochain":
                return
            stt(dst[:], ps[:, :, 0], 0.0, sb["cvec_c"][:],
                AluOp.add, AluOp.add)
            stt(dst[:], ps[:, :, 1], 0.0, dst[:], AluOp.add, AluOp.add)

        def s_accum(bj, init=False):
            """S (+)= bj * hf  (hf = final hidden of the stage)."""
            if variant == "nochain":
                return
            if init:
                nc.vector.tensor_scalar(S[:], hf[:], bj, None, AluOp.mult)
            else:
                stt(S[:], hf[:], bj, S[:])

        def full_stage(src_preact, g_dst, bj, s_init=False):
            if variant != "nochain":
                softplus_from(src_preact)
                split_x(hf, x12, x1s, CH)
            hidden_chain()
            s_accum(bj, init=s_init)
            g_stage(g_dst)

        # --- init: out[0], k1 chain (FSAL seed) ---
        nc.vector.tensor_tensor(zc[:], sb["eps_c"][:, 0:CH], ytop[:, CH:2 * CH],
                                AluOp.mult)
        nc.vector.tensor_tensor(out_sb[:, 0:CH], zc[:], ytop[:, 0:CH],
                                AluOp.add)
        if variant == "nochain":  # timing-only: static operands everywhere
            nc.vector.tensor_copy(out_sb[:], sb["eps_c"][:])
            split_x(w, x12, x1s, CH)
            split_x(ytop, S12, S1s, CH)
        full_stage(w[:], gs[0], B_COEF[0], s_init=True)

        # --- main loop ---
        def step_body(i, half=0):
            if variant == "nochain":
                for _ in range(6):
                    full_stage(w[:], gs[0], B_COEF[0])
                mv("Wt_o", S12, S1s, CH, CT, psum_big)
                return
            cur_st[0] = st_ab[half]
            nc.vector.tensor_copy(
                cur_st[0][:], sb["scal"][:, bass.ds(i * N_SCAL, N_SCAL)])
            # wacc = w + dt*B1*g'_1  (incremental w' update)
            stt(wacc[:], gs[0][:], sc(15), w[:])
            aidx = 0
            for s in range(5):  # stages k2..k6
                # zin = w + dt * sum_j A_sj g'_j
                stt(zin[:], gs[0][:], sc(aidx), w[:])
                for j in range(1, s + 1):
                    stt(zin[:], gs[j][:], sc(aidx + j), zin[:])
                aidx += s + 1
                full_stage(zin[:], gs[s + 1], B_COEF[s + 1])
                # wacc += dt*B_j*g'_j
                stt(wacc[:], gs[s + 1][:], sc(15 + s + 1), wacc[:])
            # y-top update: ytop += dt*(Wo@S) + dt*kappa*b_out_top
            # (emitted before stage k7, which re-initializes S)
            split_x(S, S12, S1s, CH)
            psb = mv("Wt_o", S12, S1s, CH, CT, psum_big)
            stt(ytop[:], sb["boK_c"][:], sc(21), ytop[:])
            stt(ytop[:], psb[:, :, 0], sc(21), ytop[:])
            stt(ytop[:], psb[:, :, 1], sc(21), ytop[:])
            # output row i+1
            eslice = sb["eps_c"][:, bass.ds((i + 1) * CH, CH)]
            oslice = out_sb[:, bass.ds((i + 1) * CH, CH)]
            nc.vector.tensor_tensor(zc[:], eslice, ytop[:, CH:2 * CH],
                                    AluOp.mult)
            nc.vector.tensor_tensor(oslice, zc[:], ytop[:, 0:CH], AluOp.add)
            # stage k7 = f(y1): preact = wacc; becomes next step's k1
            full_stage(wacc[:], gs[6], B_COEF[0], s_init=True)
            nc.vector.tensor_copy(gs[0][:], gs[6][:])
            nc.vector.tensor_copy(w[:], wacc[:])

        def main_loop():
            with tc.For_i(0, n_steps - 1, 2,
                          hint_engines=tuple(mybir.ALL_ENGINES)) as iv:
                step_body(iv, 0)
                step_body(iv + 1, 1)
            step_body(n_steps - 1, 0)

        if repeat == 1:
            main_loop()
        elif repeat > 1:  # timing builds: amplify device time
            with tc.For_i(0, repeat, 1,
                          hint_engines=tuple(mybir.ALL_ENGINES)):
                main_loop()
        # repeat == 0: no step loop (init-only timing reference)

        nc.sync.dma_start(out_ap, out_sb[:])

    try:
        nc.compile()
    finally:
        bacc.get_activation_tables = orig_tables
    return nc


def _get_nc():
    if "nc" not in _CACHE:
        _CACHE["nc"] = _build_kernel()
    return _CACHE["nc"]


def kernel(**inputs) -> np.ndarray:
    from concourse.bass_utils import run_bass_kernel_spmd

    host_in = _prep_host_inputs(inputs)
    nc = _get_nc()
    res = run_bass_kernel_spmd(nc, [host_in], core_ids=[0])
    out_c = res.results[0]["out_c"]
    out = _uncol_layout(
        out_c.reshape(128, NSTEPS, CH).transpose(1, 0, 2)).astype(np.float32)
    return out
